# revision 26
# baseline (speedup 1.0000x reference)
"""BetaTCVAE loss kernel for Trainium2 (8 NeuronCores, SPMD).

Math: for z, z_mean, z_logvar in R^[B, L] (B=4096, L=16):
  P_l[i,j] = log N(z[i,l]; mean[j,l], var[j,l])
           = A[i,l]*U[j,l] + B[i,l]*V[j,l] + W[j,l]
    with A = z^2, B = z, U = -0.5*exp(-lv), V = mean*exp(-lv),
         W = -0.5*(mean^2*exp(-lv) + lv + log(2pi))
  log_qz_product[i] = sum_l log sum_j exp(P_l[i,j])
  log_qz[i]         = log sum_j exp(sum_l P_l[i,j])
  out = (w_tc - 1) * mean_i(log_qz - log_qz_product)

Key observation: P_l[i,j] depends on i only through the scalar x = z[i,l],
so  f_l(x) = sum_j exp(P_l(x, j))  is a univariate function (a Gaussian
mixture in x). The 16 per-dim logsumexp planes therefore do NOT need the
full [B, B, L] evaluation: the device tabulates f_l on a G-point uniform
grid covering the z range (G*B*L exps total instead of B^2*L), and the
host interpolates log f_l at the B*L z values with 4-point Lagrange
(final rel err ~ 1e-5, tolerance is 2e-2). Only the summed plane
S = sum_l P_l (log_qz) genuinely needs B^2 work and stays exact.

Device strategy (8 cores):
  - Phase A (S-plane): shard rows i; per core [512 i, 4096 j] via K=96
    fp16 hi/lo matmul pairs (exact products in f32 PSUM), ScalarE Exp into
    bf16 sinks, VectorE tensor_tensor_reduce (add halves + row-sum fused).
  - Phase B (tables): shard j; per core [G grid, 512 j] per-dim planes via
    the K=12 merged hi/lo matmul (one pass), Exp, VectorE per-l row sums.
    Partial tables [G, 16] are summed across cores on the host (tiny).
  - Host (f64, O(B*L)): log of table, Lagrange interpolation, final mean.

ScalarE work/core: (B*4096 + G*512*16)/  = 2.1M + 2.1M (G=256) exps vs
35.7M for the all-on-device baseline (~7x less); ACT and DVE end up
co-bottlenecked at ~30us/core.
"""

import math
import os

# No NTFF hook exists in this container; a stray BASS_TRACE=1 would crash
# run_bass_kernel_spmd on the axon path. Force tracing off.
os.environ["BASS_NEVER_TRACE"] = "1"

import numpy as np
from contextlib import ExitStack

import concourse.bass as bass
import concourse.tile as tile
from concourse import mybir
from concourse.bass_utils import run_bass_kernel_spmd

F32 = mybir.dt.float32
F16 = mybir.dt.float16
BF16 = mybir.dt.bfloat16
EXP = mybir.ActivationFunctionType.Exp

B = 4096
L = 16
N_CORES = 8
I_PER_CORE = B // N_CORES          # 512
N_ITILES = I_PER_CORE // 128       # 4
J_PER_CORE = B // N_CORES          # table j-shard per core
G = 64                             # grid points; 2 dims packed per 128-row tile
N_BSPANS = L // 8                  # phase-B spans: 8 dims (4 pairs) per span
CHUNK = 512                        # matmul N (1 PSUM bank)
HALF = 2048                        # ACT span (4 PSUM banks)
NACC_A = N_ITILES                  # one S row-sum col per i-tile
NACC = NACC_A + N_BSPANS * 4       # + one table col per (span, pair-chunk)
W_TC = 2.0
LOG_2PI = math.log(2.0 * math.pi)

_CACHE = {}


def _split_f16(x):
    hi = x.astype(np.float16)
    lo = (x - hi.astype(np.float64)).astype(np.float16)
    return hi, lo


def _split_multi_waits(nc, keep: int = 1) -> int:
    """This walrus build rejects >1 embedded sem wait per instruction.
    Hoist extras onto standalone same-engine NoOps placed just before."""
    n_split = 0
    for f in nc.m.functions:
        for blk in f.blocks:
            insts = blk.instructions
            if not any(
                i.sync_info is not None and len(i.sync_info.on_wait) > keep
                for i in insts
            ):
                continue
            out = []
            for inst in insts:
                si = inst.sync_info
                if si is not None and len(si.on_wait) > keep:
                    waits = list(si.on_wait)
                    for w in waits[:-keep]:
                        nop = mybir.InstNoOp(
                            name=f"{inst.name}_wsplit{n_split}",
                            ins=[],
                            outs=[],
                            text_hint="split_wait",
                            bass_nofuse=True,
                        )
                        nop.engine = inst.engine
                        nop.sync_info = mybir.SyncInfo(on_wait=[w], on_update=[])
                        out.append(nop)
                        n_split += 1
                    inst.sync_info = mybir.SyncInfo(
                        on_wait=waits[-keep:], on_update=list(si.on_update)
                    )
                out.append(inst)
            blk.instructions = out
    return n_split


def _build_nc(reps: int = 1, sink_bufs: int = 8, order: str = "mix", pool_adds: int = 4):
    """reps=1: the real kernel. reps>1: same compute wrapped in a hardware
    For_i loop (benchmark mode - device time dominates wall-clock).
    reps<0: python-unrolled |reps| copies, for TimelineSim steady-state."""
    nc = bass.Bass()
    ltSa_d = nc.declare_dram_parameter("ltSa", [96, N_ITILES * 128], F16, isOutput=False)
    ltSb_d = nc.declare_dram_parameter("ltSb", [96, N_ITILES * 128], F16, isOutput=False)
    rhsS_d = nc.declare_dram_parameter("rhsS", [96, B], F16, isOutput=False)
    gridlt_d = nc.declare_dram_parameter("gridlt", [128, 128], F16, isOutput=False)
    gridrhs_d = nc.declare_dram_parameter("gridrhs", [128, N_BSPANS * CHUNK], F16, isOutput=False)
    acc_d = nc.declare_dram_parameter("acc", [128, NACC], F32, isOutput=True)

    with tile.TileContext(nc) as tc, ExitStack() as ctx:
        const = ctx.enter_context(tc.tile_pool(name="const", bufs=1))
        psum = ctx.enter_context(tc.tile_pool(name="psum", bufs=2, space="PSUM"))
        sink_pool = ctx.enter_context(tc.tile_pool(name="sink", bufs=sink_bufs))

        ltSa = const.tile([96, N_ITILES * 128], F16)
        nc.sync.dma_start(ltSa[:], ltSa_d[:])
        ltSb = const.tile([96, N_ITILES * 128], F16)
        nc.sync.dma_start(ltSb[:], ltSb_d[:])
        rhsS = const.tile([96, B], F16)
        nc.sync.dma_start(rhsS[:], rhsS_d[:])
        gridlt = const.tile([128, 128], F16)
        nc.sync.dma_start(gridlt[:], gridlt_d[:])
        gridrhs = const.tile([128, N_BSPANS * CHUNK], F16)
        nc.sync.dma_start(gridrhs[:], gridrhs_d[:])

        acc = const.tile([128, NACC], F32)

        # ACT table warmup: first Exp carries the table load; give it one dep.
        warm = const.tile([128, 1], F32)
        nc.vector.memset(warm[:], 0.0)
        nc.scalar.activation(warm[:], warm[:], EXP)

        def body_A(t_list):
            # Phase A: S-plane row sums. Per i-tile t: two j-half spans of
            # [128, 2048], exp'd to bf16 sinks, added, row-sum-reduced into
            # acc[:, t].
            for t in t_list:
                sinks = []
                for h in range(2):
                    ps = psum.tile([128, 4, CHUNK], F32, tag="ps")
                    # a,a,a,a then b,b,b,b: lhsT changes once per span
                    for c in range(4):
                        j0 = h * HALF + c * CHUNK
                        nc.tensor.matmul(
                            ps[:, c, :],
                            ltSa[:, t * 128 : (t + 1) * 128],
                            rhsS[:, j0 : j0 + CHUNK],
                            start=True, stop=False, tile_position=(0, 0),
                        )
                    for c in range(4):
                        j0 = h * HALF + c * CHUNK
                        nc.tensor.matmul(
                            ps[:, c, :],
                            ltSb[:, t * 128 : (t + 1) * 128],
                            rhsS[:, j0 : j0 + CHUNK],
                            start=False, stop=True, tile_position=(0, 0),
                        )
                    sink = sink_pool.tile([128, 4, CHUNK], BF16, tag="sink")
                    nc.scalar.activation(sink[:, :, :], ps[:, :, :], EXP)
                    sinks.append(sink)
                # half-add: alternate between the otherwise-idle Pool engine
                # and DVE (Pool is ~4x slower per elem; a 50/50 split keeps
                # both well under ACT's span rate). Row-sum on DVE.
                add_eng = nc.gpsimd if t < pool_adds else nc.vector
                add_eng.tensor_add(
                    sinks[0][:, :, :], sinks[0][:, :, :], sinks[1][:, :, :]
                )
                nc.vector.tensor_reduce(
                    acc[:, t : t + 1],
                    sinks[0][:, :, :],
                    axis=mybir.AxisListType.XY,
                    op=mybir.AluOpType.add,
                )

        def body_B(s_list):
            # Phase B: per-dim grid tables, 2 dims packed per 128-partition
            # tile (grid rows 0..63 -> even dim, 64..127 -> odd dim) via K=24
            # block-diagonal lhsT in a 32-row quadrant band. Span s covers
            # pairs p = 4s+c (dims 2p, 2p+1) over the core's 512-j shard.
            for s in s_list:
                ps = psum.tile([128, 4, CHUNK], F32, tag="ps")
                for c in range(4):
                    nc.tensor.matmul(
                        ps[:, c, :],
                        gridlt[32 * c : 32 * c + 24, :],
                        gridrhs[32 * c : 32 * c + 24, s * CHUNK : (s + 1) * CHUNK],
                        start=True, stop=True, tile_position=(32 * c, 0),
                    )
                sink = sink_pool.tile([128, 4, CHUNK], BF16, tag="sink")
                nc.scalar.activation(sink[:, :, :], ps[:, :, :], EXP)
                col = NACC_A + s * 4
                nc.vector.tensor_reduce(
                    acc[:, col : col + 4],
                    sink[:, :, :],
                    axis=mybir.AxisListType.X,
                    op=mybir.AluOpType.add,
                )

        def body():
            if order == "AB":
                body_A(range(N_ITILES)); body_B(range(N_BSPANS))
            elif order == "BA":
                body_B(range(N_BSPANS)); body_A(range(N_ITILES))
            elif order == "ABA":  # B spans sandwiched mid-A
                body_A(range(2)); body_B(range(N_BSPANS)); body_A(range(2, N_ITILES))
            elif order == "mix":  # one B span after each half of A
                body_A(range(2)); body_B(range(1))
                body_A(range(2, N_ITILES)); body_B(range(1, N_BSPANS))

        if reps == 1:
            body()
        elif reps < 0:  # python-unrolled, for TimelineSim steady-state reads
            for _ in range(-reps):
                body()
        else:
            with tc.For_i(0, reps, 1):
                body()

        nc.sync.dma_start(acc_d[:], acc[:])

    _split_multi_waits(nc)
    return nc


def _grid_params(z):
    z = np.asarray(z, np.float64)
    lo, hi = float(z.min()), float(z.max())
    h = max(hi - lo, 1e-3) / (G - 7)
    g0 = lo - 3.0 * h
    return g0, h


def _pack_inputs(z, z_mean, z_logvar):
    """Build per-core input maps (float64 host math, fp16 hi/lo splits)."""
    z = np.asarray(z, np.float64)
    mean = np.asarray(z_mean, np.float64)
    lv = np.asarray(z_logvar, np.float64)

    iv = np.exp(-lv)
    U = -0.5 * iv                                   # [B, L]
    V = mean * iv
    W = -0.5 * (mean * mean * iv + lv + LOG_2PI)
    A = z * z
    Bz = z

    Uh, Ul = _split_f16(U)
    Vh, Vl = _split_f16(V)
    Wh, Wl = _split_f16(W)
    Ah, Al = _split_f16(A)
    Bh, Bl = _split_f16(Bz)

    g0, h = _grid_params(z)
    grid = g0 + h * np.arange(G)
    Gh, Gl = _split_f16(grid)            # B-coefficient of grid rows
    G2h, G2l = _split_f16(grid * grid)   # A-coefficient

    # gridlt [128, 128]: per quadrant band c, a K=24 block-diagonal lhsT:
    # rows 32c+{0..11} = K=12-merged grid block on partitions 0..63 (even
    # dim of the pair), rows 32c+{12..23} = same block on 64..127 (odd dim).
    ones = np.ones(G, np.float16)
    zer = np.zeros(G, np.float16)
    block = np.stack([G2h, Gh, ones, G2l, Gl, zer] * 2)  # [12, G=64]
    gridlt = np.zeros((128, 128), np.float16)
    for c in range(4):
        gridlt[32 * c : 32 * c + 12, :G] = block
        gridlt[32 * c + 12 : 32 * c + 24, G:] = block

    in_maps = []
    onesB, zerB = np.ones(128, np.float16), np.zeros(128, np.float16)
    for c in range(N_CORES):
        # S-plane lhsT pair: a = [Hi_w; Lo_w], b = [Lo_w; Hi_w]
        ltSa = np.zeros((96, N_ITILES * 128), np.float16)
        for t in range(N_ITILES):
            rows = slice(512 * c + 128 * t, 512 * c + 128 * (t + 1))
            col = slice(t * 128, (t + 1) * 128)
            for l in range(L):
                ltSa[3 * l + 0, col] = Ah[rows, l]
                ltSa[3 * l + 1, col] = Bh[rows, l]
                ltSa[3 * l + 2, col] = onesB
                ltSa[48 + 3 * l + 0, col] = Al[rows, l]
                ltSa[48 + 3 * l + 1, col] = Bl[rows, l]
                ltSa[48 + 3 * l + 2, col] = zerB
        ltSb = np.concatenate([ltSa[48:], ltSa[:48]], axis=0)

        # S-plane rhs: rows [Hi_r(48); Lo_r(48)], all B j's (replicated).
        # gridrhs: per-dim U,V,W hi/lo for the core's j-shard, K=12 layout.
        if c == 0:
            rhsS = np.zeros((96, B), np.float16)
            for l in range(L):
                rhsS[3 * l + 0] = Uh[:, l]
                rhsS[3 * l + 1] = Vh[:, l]
                rhsS[3 * l + 2] = Wh[:, l]
                rhsS[48 + 3 * l + 0] = Ul[:, l]
                rhsS[48 + 3 * l + 1] = Vl[:, l]
                rhsS[48 + 3 * l + 2] = Wl[:, l]

        # gridrhs: pair p = 4s+cc (dims 2p, 2p+1) lives at band rows 32cc,
        # cols s*512; rows +0..11 = even dim's merged U,V,W rows (pairs with
        # the K=12 grid block), rows +12..23 = odd dim's.
        jsh = slice(J_PER_CORE * c, J_PER_CORE * (c + 1))
        gridrhs = np.zeros((128, N_BSPANS * J_PER_CORE), np.float16)
        for p in range(L // 2):
            s, cc = p >> 2, p & 3
            cols = slice(s * J_PER_CORE, (s + 1) * J_PER_CORE)
            for half, l in enumerate((2 * p, 2 * p + 1)):
                r0 = 32 * cc + 12 * half
                for k, (h_, lo_) in enumerate([(Uh, Ul), (Vh, Vl), (Wh, Wl)]):
                    gridrhs[r0 + k, cols] = h_[jsh, l]
                    gridrhs[r0 + 3 + k, cols] = h_[jsh, l]
                    gridrhs[r0 + 6 + k, cols] = lo_[jsh, l]
                    gridrhs[r0 + 9 + k, cols] = lo_[jsh, l]

        in_maps.append({
            "ltSa": ltSa, "ltSb": ltSb, "rhsS": rhsS,
            "gridlt": gridlt, "gridrhs": gridrhs,
        })
    return in_maps


LAST_RESULT = None


def kernel(z, z_mean, z_logvar):
    global LAST_RESULT
    if "nc" not in _CACHE:
        _CACHE["nc"] = _build_nc()
    nc = _CACHE["nc"]
    in_maps = _pack_inputs(z, z_mean, z_logvar)
    res = run_bass_kernel_spmd(nc, in_maps, list(range(N_CORES)))
    LAST_RESULT = res

    # Host reduction in float64.
    z64 = np.asarray(z, np.float64)
    g0, h = _grid_params(z64)

    # S-plane: acc[p, t] on core c = sum_j exp(S[i, j]) for i = 512c+128t+p
    sums_S = np.zeros(B)
    ftab = np.zeros((G, L))
    for c in range(N_CORES):
        acc = np.asarray(res.results[c]["acc"], np.float64)
        for t in range(N_ITILES):
            sums_S[512 * c + 128 * t : 512 * c + 128 * (t + 1)] = acc[:, t]
        # tables: col NACC_A + s*4 + cc <-> pair p = 4s+cc; partitions 0..63
        # hold dim 2p over the grid, 64..127 hold dim 2p+1.
        for p in range(L // 2):
            s, cc = p >> 2, p & 3
            col = NACC_A + s * 4 + cc
            ftab[:, 2 * p] += acc[:G, col]
            ftab[:, 2 * p + 1] += acc[G : 2 * G, col]
    log_qz = np.log(sums_S)

    gtab = np.log(ftab)  # [G, L]
    t = (z64 - g0) / h
    i0 = np.clip(np.floor(t).astype(int), 1, G - 3)
    f = t - i0
    w0 = -f * (f - 1) * (f - 2) / 6
    w1 = (f + 1) * (f - 1) * (f - 2) / 2
    w2 = -(f + 1) * f * (f - 2) / 2
    w3 = (f + 1) * f * (f - 1) / 6
    cols = np.arange(L)[None, :].repeat(B, 0)
    lqp = (w0 * gtab[i0 - 1, cols] + w1 * gtab[i0, cols]
           + w2 * gtab[i0 + 1, cols] + w3 * gtab[i0 + 2, cols]).sum(axis=1)

    out = (W_TC - 1.0) * float(np.mean(log_qz - lqp))
    return np.float32(out)


# revision 42
# speedup vs baseline: 3.0185x; 3.0185x over previous
"""BetaTCVAE loss kernel for Trainium2 (8 NeuronCores, SPMD).

Math: for z, z_mean, z_logvar in R^[B, L] (B=4096, L=16):
  P_l[i,j] = log N(z[i,l]; mean[j,l], var[j,l])
           = A[i,l]*U[j,l] + B[i,l]*V[j,l] + W[j,l]
    with A = z^2, B = z, U = -0.5*exp(-lv), V = mean*exp(-lv),
         W = -0.5*(mean^2*exp(-lv) + lv + log(2pi))
  log_qz_product[i] = sum_l log sum_j exp(P_l[i,j])
  log_qz[i]         = log sum_j exp(sum_l P_l[i,j])
  out = (w_tc - 1) * mean_i(log_qz - log_qz_product)

Key observation: P_l[i,j] depends on i only through the scalar x = z[i,l],
so  f_l(x) = sum_j exp(P_l(x, j))  is a univariate function (a Gaussian
mixture in x). The 16 per-dim logsumexp planes therefore do NOT need the
full [B, B, L] evaluation: the device tabulates f_l on a G-point uniform
grid covering the z range (G*B*L exps total instead of B^2*L), and the
host interpolates log f_l at the B*L z values with 4-point Lagrange
(final rel err ~ 1e-5, tolerance is 2e-2). Only the summed plane
S = sum_l P_l (log_qz) genuinely needs B^2 work and stays exact.

Device strategy (8 cores):
  - Phase A (S-plane): shard rows i; per core [512 i, 4096 j] via K=96
    fp16 hi/lo matmul pairs (exact products in f32 PSUM), ScalarE Exp into
    bf16 sinks, VectorE tensor_tensor_reduce (add halves + row-sum fused).
  - Phase B (tables): shard j; per core [G grid, 512 j] per-dim planes via
    the K=12 merged hi/lo matmul (one pass), Exp, VectorE per-l row sums.
    Partial tables [G, 16] are summed across cores on the host (tiny).
  - Host (f64, O(B*L)): log of table, Lagrange interpolation, final mean.

ScalarE work/core: (B*4096 + G*512*16)/  = 2.1M + 2.1M (G=256) exps vs
35.7M for the all-on-device baseline (~7x less); ACT and DVE end up
co-bottlenecked at ~30us/core.
"""

import math
import os

# No NTFF hook exists in this container; a stray BASS_TRACE=1 would crash
# run_bass_kernel_spmd on the axon path. Force tracing off.
os.environ["BASS_NEVER_TRACE"] = "1"

import numpy as np
from contextlib import ExitStack

import concourse.bass as bass
import concourse.tile as tile
from concourse import mybir
from concourse.bass_utils import run_bass_kernel_spmd

F32 = mybir.dt.float32
F16 = mybir.dt.float16
BF16 = mybir.dt.bfloat16
EXP = mybir.ActivationFunctionType.Exp

B = 4096
L = 16
N_CORES = 8
I_PER_CORE = B // N_CORES          # 512
N_ITILES = I_PER_CORE // 128       # 4
J_PER_CORE = B // N_CORES          # table j-shard per core
G = 32                             # grid points; 4 dims packed per 128-row tile
N_BSPANS = L // 16                 # phase-B spans: 16 dims (4 quads) per span
CHUNK = 512                        # matmul N (1 PSUM bank)
HALF = 2048                        # ACT span (4 PSUM banks)
NACC_A = N_ITILES                  # one S row-sum col per i-tile
NACC = NACC_A + N_BSPANS * 4       # + one table col per (span, pair-chunk)
W_TC = 2.0
LOG_2PI = math.log(2.0 * math.pi)

_CACHE = {}


def _split_f16(x):
    hi = x.astype(np.float16)
    lo = (x - hi.astype(np.float64)).astype(np.float16)
    return hi, lo


def _split_multi_waits(nc, keep: int = 1) -> int:
    """This walrus build rejects >1 embedded sem wait per instruction.
    Hoist extras onto standalone same-engine NoOps placed just before."""
    n_split = 0
    for f in nc.m.functions:
        for blk in f.blocks:
            insts = blk.instructions
            if not any(
                i.sync_info is not None and len(i.sync_info.on_wait) > keep
                for i in insts
            ):
                continue
            out = []
            for inst in insts:
                si = inst.sync_info
                if si is not None and len(si.on_wait) > keep:
                    waits = list(si.on_wait)
                    for w in waits[:-keep]:
                        nop = mybir.InstNoOp(
                            name=f"{inst.name}_wsplit{n_split}",
                            ins=[],
                            outs=[],
                            text_hint="split_wait",
                            bass_nofuse=True,
                        )
                        nop.engine = inst.engine
                        nop.sync_info = mybir.SyncInfo(on_wait=[w], on_update=[])
                        out.append(nop)
                        n_split += 1
                    inst.sync_info = mybir.SyncInfo(
                        on_wait=waits[-keep:], on_update=list(si.on_update)
                    )
                out.append(inst)
            blk.instructions = out
    return n_split


def _build_nc(reps: int = 1, sink_bufs: int = 8, order: str = "AB", pool_adds: int = 0,
              fold: bool = True):
    """reps=1: the real kernel. reps>1: same compute wrapped in a hardware
    For_i loop (benchmark mode - device time dominates wall-clock).
    reps<0: python-unrolled |reps| copies, for TimelineSim steady-state."""
    nc = bass.Bass()
    # S-plane, single K=128 pass: lhsT rows [Ah,Bh,1 | Ah,Bh,1 | Al,Bl],
    # rhs rows [Uh,Vh,Wh | Ul,Vl,Wl | Uh,Vh] -> hh + hl + lh products
    # (the lo*lo term, ~1e-7 relative, is dropped).
    ltS_d = nc.declare_dram_parameter("ltS", [128, N_ITILES * 128], F16, isOutput=False)
    rhsS_d = nc.declare_dram_parameter("rhsS", [128, B], F16, isOutput=False)
    gridlt_d = nc.declare_dram_parameter("gridlt", [128, 128], F16, isOutput=False)
    gridrhs_d = nc.declare_dram_parameter("gridrhs", [128, 2 * CHUNK], F16, isOutput=False)
    acc_d = nc.declare_dram_parameter("acc", [128, NACC], F32, isOutput=True)

    with tile.TileContext(nc) as tc, ExitStack() as ctx:
        const = ctx.enter_context(tc.tile_pool(name="const", bufs=1))
        psum = ctx.enter_context(tc.tile_pool(name="psum", bufs=2, space="PSUM"))
        sink_pool = ctx.enter_context(tc.tile_pool(name="sink", bufs=sink_bufs))

        ltS = const.tile([128, N_ITILES * 128], F16)
        nc.sync.dma_start(ltS[:], ltS_d[:])
        rhsS = const.tile([128, B], F16)
        nc.sync.dma_start(rhsS[:], rhsS_d[:])
        gridlt = const.tile([128, 128], F16)
        nc.sync.dma_start(gridlt[:], gridlt_d[:])
        gridrhs = const.tile([128, 2 * CHUNK], F16)
        nc.sync.dma_start(gridrhs[:], gridrhs_d[:])

        acc = const.tile([128, NACC], F32)

        # ACT table warmup: first Exp carries the table load; give it one dep.
        warm = const.tile([128, 1], F32)
        nc.vector.memset(warm[:], 0.0)
        nc.scalar.activation(warm[:], warm[:], EXP)

        def body_A(t_list):
            # Phase A: S-plane row sums. Per i-tile t: two j-half spans of
            # [128, 2048], exp'd to bf16 sinks, added, row-sum-reduced into
            # acc[:, t].
            for t in t_list:
                sinks = []
                for h in range(2):
                    ps = psum.tile([128, 4, CHUNK], F32, tag="ps")
                    for c in range(4):
                        j0 = h * HALF + c * CHUNK
                        nc.tensor.matmul(
                            ps[:, c, :],
                            ltS[:, t * 128 : (t + 1) * 128],
                            rhsS[:, j0 : j0 + CHUNK],
                            start=True, stop=True, tile_position=(0, 0),
                        )
                    sink = sink_pool.tile([128, 4, CHUNK], BF16, tag="sink")
                    nc.scalar.activation(sink[:, :, :], ps[:, :, :], EXP)
                    sinks.append(sink)
                # Row-sum chain on DVE: bf16 adds run at 2x but reduce at 1x,
                # so fold once before the reduce (2133 -> 1067+533+1067 over
                # half the elems each).
                add_eng = nc.gpsimd if t < pool_adds else nc.vector
                add_eng.tensor_add(
                    sinks[0][:, :, :], sinks[0][:, :, :], sinks[1][:, :, :]
                )
                if fold:
                    nc.vector.tensor_add(
                        sinks[0][:, 0:2, :], sinks[0][:, 0:2, :], sinks[0][:, 2:4, :]
                    )
                red_in = sinks[0][:, 0:2, :] if fold else sinks[0][:, :, :]
                nc.vector.tensor_reduce(
                    acc[:, t : t + 1],
                    red_in,
                    axis=mybir.AxisListType.XY,
                    op=mybir.AluOpType.add,
                )

        def body_B(s_list):
            # Phase B: per-dim grid tables, 4 dims packed per 128-partition
            # tile (partition 32d+g -> dim-in-quad d, grid point g) via K=48
            # block-diagonal lhsT in a 64-row half band. Chunk c covers dims
            # 4c..4c+3 over the core's 512-j shard, placed at (band 64*(c&1),
            # rhs col block c>>1). One span total.
            for s in s_list:
                ps = psum.tile([128, 4, CHUNK], F32, tag="ps")
                for c in range(4):
                    band = 64 * (c & 1)
                    blk = (c >> 1) * CHUNK
                    nc.tensor.matmul(
                        ps[:, c, :],
                        gridlt[band : band + 48, :],
                        gridrhs[band : band + 48, blk : blk + CHUNK],
                        start=True, stop=True, tile_position=(band, 0),
                    )
                sink = sink_pool.tile([128, 4, CHUNK], BF16, tag="sink")
                nc.scalar.activation(sink[:, :, :], ps[:, :, :], EXP)
                col = NACC_A + s * 4
                nc.vector.tensor_reduce(
                    acc[:, col : col + 4],
                    sink[:, :, :],
                    axis=mybir.AxisListType.X,
                    op=mybir.AluOpType.add,
                )

        def body():
            if order == "AB":
                body_A(range(N_ITILES)); body_B(range(N_BSPANS))
            elif order == "BA":
                body_B(range(N_BSPANS)); body_A(range(N_ITILES))
            elif order == "ABA":  # B spans sandwiched mid-A
                body_A(range(2)); body_B(range(N_BSPANS)); body_A(range(2, N_ITILES))
            elif order == "mix":  # one B span after each half of A
                body_A(range(2)); body_B(range(1))
                body_A(range(2, N_ITILES)); body_B(range(1, N_BSPANS))

        if reps == 1:
            body()
        elif reps < 0:  # python-unrolled, for TimelineSim steady-state reads
            for _ in range(-reps):
                body()
        else:
            with tc.For_i(0, reps, 1):
                body()

        nc.sync.dma_start(acc_d[:], acc[:])

    _split_multi_waits(nc)
    return nc


def _grid_params(z):
    z = np.asarray(z, np.float64)
    lo, hi = float(z.min()), float(z.max())
    h = max(hi - lo, 1e-3) / (G - 7)
    g0 = lo - 3.0 * h
    return g0, h


def _pack_inputs(z, z_mean, z_logvar):
    """Build per-core input maps (float64 host math, fp16 hi/lo splits)."""
    z = np.asarray(z, np.float64)
    mean = np.asarray(z_mean, np.float64)
    lv = np.asarray(z_logvar, np.float64)

    iv = np.exp(-lv)
    U = -0.5 * iv                                   # [B, L]
    V = mean * iv
    W = -0.5 * (mean * mean * iv + lv + LOG_2PI)
    A = z * z
    Bz = z

    Uh, Ul = _split_f16(U)
    Vh, Vl = _split_f16(V)
    Wh, Wl = _split_f16(W)
    Ah, Al = _split_f16(A)
    Bh, Bl = _split_f16(Bz)

    g0, h = _grid_params(z)
    grid = g0 + h * np.arange(G)
    Gh, Gl = _split_f16(grid)            # B-coefficient of grid rows
    G2h, G2l = _split_f16(grid * grid)   # A-coefficient

    # gridlt [128, 128]: two identical K=48 half-band blocks (rows 0..47 and
    # 64..111); within a band, rows 12d+{0..11} = K=12-merged grid block on
    # partitions 32d..32d+31 (dim-in-quad d).
    ones = np.ones(G, np.float16)
    zer = np.zeros(G, np.float16)
    block = np.stack([G2h, Gh, ones, G2l, Gl, zer] * 2)  # [12, G=32]
    gridlt = np.zeros((128, 128), np.float16)
    for band in (0, 64):
        for d in range(4):
            gridlt[band + 12 * d : band + 12 * d + 12, 32 * d : 32 * d + 32] = block

    in_maps = []
    onesB = np.ones(128, np.float16)
    for c in range(N_CORES):
        # S-plane K=128 single-pass layout (hh + hl + lh, ll dropped):
        # lhsT rows 0-47 [Ah,Bh,1], 48-95 [Ah,Bh,1], 96-127 [Al,Bl]
        ltS = np.zeros((128, N_ITILES * 128), np.float16)
        for t in range(N_ITILES):
            rows = slice(512 * c + 128 * t, 512 * c + 128 * (t + 1))
            col = slice(t * 128, (t + 1) * 128)
            for l in range(L):
                for base in (0, 48):
                    ltS[base + 3 * l + 0, col] = Ah[rows, l]
                    ltS[base + 3 * l + 1, col] = Bh[rows, l]
                    ltS[base + 3 * l + 2, col] = onesB
                ltS[96 + 2 * l + 0, col] = Al[rows, l]
                ltS[96 + 2 * l + 1, col] = Bl[rows, l]

        # S-plane rhs rows 0-47 [Uh,Vh,Wh], 48-95 [Ul,Vl,Wl], 96-127 [Uh,Vh]
        if c == 0:
            rhsS = np.zeros((128, B), np.float16)
            for l in range(L):
                rhsS[3 * l + 0] = Uh[:, l]
                rhsS[3 * l + 1] = Vh[:, l]
                rhsS[3 * l + 2] = Wh[:, l]
                rhsS[48 + 3 * l + 0] = Ul[:, l]
                rhsS[48 + 3 * l + 1] = Vl[:, l]
                rhsS[48 + 3 * l + 2] = Wl[:, l]
                rhsS[96 + 2 * l + 0] = Uh[:, l]
                rhsS[96 + 2 * l + 1] = Vh[:, l]

        # gridrhs: chunk cc (dims 4cc..4cc+3) at band rows 64*(cc&1), col
        # block (cc>>1)*512; rows band+12d+{0..11} = merged U,V,W rows of
        # dim 4cc+d (pairs with the K=12 grid block in gridlt).
        jsh = slice(J_PER_CORE * c, J_PER_CORE * (c + 1))
        gridrhs = np.zeros((128, 2 * J_PER_CORE), np.float16)
        for cc in range(4):
            band, blk = 64 * (cc & 1), (cc >> 1) * J_PER_CORE
            cols = slice(blk, blk + J_PER_CORE)
            for d in range(4):
                l = 4 * cc + d
                r0 = band + 12 * d
                for k, (h_, lo_) in enumerate([(Uh, Ul), (Vh, Vl), (Wh, Wl)]):
                    gridrhs[r0 + k, cols] = h_[jsh, l]
                    gridrhs[r0 + 3 + k, cols] = h_[jsh, l]
                    gridrhs[r0 + 6 + k, cols] = lo_[jsh, l]
                    gridrhs[r0 + 9 + k, cols] = lo_[jsh, l]

        in_maps.append({
            "ltS": ltS, "rhsS": rhsS,
            "gridlt": gridlt, "gridrhs": gridrhs,
        })
    return in_maps


LAST_RESULT = None


def kernel(z, z_mean, z_logvar):
    global LAST_RESULT
    if "nc" not in _CACHE:
        _CACHE["nc"] = _build_nc()
    nc = _CACHE["nc"]
    in_maps = _pack_inputs(z, z_mean, z_logvar)
    res = run_bass_kernel_spmd(nc, in_maps, list(range(N_CORES)))
    LAST_RESULT = res

    # Host reduction in float64.
    z64 = np.asarray(z, np.float64)
    g0, h = _grid_params(z64)

    # S-plane: acc[p, t] on core c = sum_j exp(S[i, j]) for i = 512c+128t+p
    sums_S = np.zeros(B)
    ftab = np.zeros((G, L))
    for c in range(N_CORES):
        acc = np.asarray(res.results[c]["acc"], np.float64)
        for t in range(N_ITILES):
            sums_S[512 * c + 128 * t : 512 * c + 128 * (t + 1)] = acc[:, t]
        # tables: col NACC_A + cc <-> dims 4cc..4cc+3; partition 32d+g holds
        # dim 4cc+d at grid point g.
        for cc in range(4):
            col = NACC_A + cc
            for d in range(4):
                ftab[:, 4 * cc + d] += acc[32 * d : 32 * d + G, col]
    log_qz = np.log(sums_S)

    gtab = np.log(ftab)  # [G, L]
    t = (z64 - g0) / h
    i0 = np.clip(np.floor(t).astype(int), 1, G - 3)
    f = t - i0
    w0 = -f * (f - 1) * (f - 2) / 6
    w1 = (f + 1) * (f - 1) * (f - 2) / 2
    w2 = -(f + 1) * f * (f - 2) / 2
    w3 = (f + 1) * f * (f - 1) / 6
    cols = np.arange(L)[None, :].repeat(B, 0)
    lqp = (w0 * gtab[i0 - 1, cols] + w1 * gtab[i0, cols]
           + w2 * gtab[i0 + 1, cols] + w3 * gtab[i0 + 2, cols]).sum(axis=1)

    out = (W_TC - 1.0) * float(np.mean(log_qz - lqp))
    return np.float32(out)
